# revision 28
# baseline (speedup 1.0000x reference)
"""Trainium2 Bass/Tile kernel for nn_MirrorAggregator.

Math (per batch, N=256 nodes, D=128 dim):
  alpha[n] = scale * s[n,:] @ (Wq1^T Wk1) @ m[n,:]^T
  sat_out  = s + alpha * (m - s)
  beta     = scale * (m @ (Wq2^T Wk2)) @ sat_out^T   (masked softmax over j)
  mir_out  = softmax(beta) @ m

Host folds each weight pair into one DxD constant (scale included):
  At = scale * Wk1^T @ Wq1    (v = m @ At, alpha = rowsum(v * s))
  Hs = scale * Wq2^T @ Wk2    (wT = Hs^T @ mT)

v5 design:
 - Pure data parallel: 64 batches per core on 8 cores; 8 batches per DMA
   chunk, loads prefetched two chunks ahead so stores never head-of-line
   block loads on the SP queue.
 - All-bf16 PE, fp32 PSUM. The host ships BOTH layouts of m as bf16
   SBUF-image tensors ([m|1] for the mir GEMM / diff, m^T for the wT and
   gate GEMMs) so the kernel never transposes m on-chip. sat_out is
   transposed on the PE (bf16 identity, is_transpose) since it is
   computed on-chip.
 - sat/mir outputs are written bf16 into SBUF-image dram tensors; the
   mir numerator and the softmax denominator (ones column of [m|1]
   riding the mir GEMM) are shipped separately and the host divides.
 - The -1e30 mask bias is accumulated into the betaT PSUM by a rank-1
   matmul (biasR row x ones row) so exp needs no bias operand.
 - Software-pipelined emission at batch-pair granularity, stages one
   pair-iteration apart:
     S1(p): diff = m - s (Pool)
     S2(p): wT GEMM + evac (Act), gate GEMM (PE), alpha dots (DVE)
     S2b(p): sat writeback (DVE)
     S3(p): satT transposes (PE) + evac (DVE)
     S4(p): betaT + bias GEMMs (PE), exp per half-pair (Act)
     S5(p): mir GEMMs (PE), numerator copies (DVE/Act) + den copies (Act)
 - GPSIMD can only run tensor_tensor/copy/memset on SBUF, so Pool holds
   just the diff; every PSUM access lives on DVE/Act, balanced ~equally.
"""

import math
import os
import sys

import numpy as np

for _p in ("/opt/trn_rl_repo",):
    if os.path.isdir(_p) and _p not in sys.path:
        sys.path.insert(0, _p)

import ml_dtypes

import concourse.bacc as bacc
import concourse.tile as tile
from concourse import mybir
from concourse.bass_utils import run_bass_kernel_spmd
from concourse.masks import make_identity

B, N, D = 512, 256, 128
NCORES = 8
BL = B // NCORES          # batches per core
ROWS = BL * N             # rows of node data per core
CH = 8                    # batches per DMA chunk
NCHUNK = BL // CH
NBLK = CH * 2             # 128-row blocks per chunk
NPAIR = BL // 2           # batch pairs per core
PPC = CH // 2             # pairs per chunk
NEG = -1.0e30
F32 = mybir.dt.float32
F32R = mybir.dt.float32r
BF16 = mybir.dt.bfloat16
BF = ml_dtypes.bfloat16

_CACHE = {}

mult = mybir.AluOpType.mult
add = mybir.AluOpType.add
sub = mybir.AluOpType.subtract
Exp = mybir.ActivationFunctionType.Exp


def _build(bl=BL):
    assert bl % CH == 0
    nchunk = bl // CH
    npair = bl // 2
    rows = bl * N
    nc = bacc.Bacc(None, target_bir_lowering=False)
    m_d = nc.declare_dram_parameter("m_img", [128, nchunk * NBLK * (D + 1)], BF16,
                                    isOutput=False)
    mt_d = nc.declare_dram_parameter("mt_img", [128, nchunk * CH * N], F32R,
                                     isOutput=False)
    s_d = nc.declare_dram_parameter("s", [rows, D], F32, isOutput=False)
    bias_d = nc.declare_dram_parameter("biasR", [2, bl * 128], BF16, isOutput=False)
    at_d = nc.declare_dram_parameter("At", [D, D], F32, isOutput=False)
    hs_d = nc.declare_dram_parameter("Hs", [D, D], F32, isOutput=False)
    sat_d = nc.declare_dram_parameter("sat_img", [128, nchunk * NBLK * D], BF16,
                                      isOutput=True)
    mir_d = nc.declare_dram_parameter("mir_img", [128, nchunk * NBLK * (D + 1)],
                                      BF16, isOutput=True)

    with tile.TileContext(nc) as tc:
        with (
            tc.tile_pool(name="const", bufs=1) as const,
            tc.tile_pool(name="mst", bufs=3) as mst,
            tc.tile_pool(name="mtst", bufs=3) as mtst,
            tc.tile_pool(name="sst", bufs=3) as sst,
            tc.tile_pool(name="ost", bufs=3) as ost,
            tc.tile_pool(name="wk", bufs=6) as wk,
            tc.tile_pool(name="ps_tp", bufs=1, space="PSUM") as ps_tp,
            tc.tile_pool(name="ps_wp", bufs=1, space="PSUM") as ps_wp,
            tc.tile_pool(name="ps_vp", bufs=2, space="PSUM") as ps_vp,
            tc.tile_pool(name="ps_bp", bufs=2, space="PSUM") as ps_bp,
            tc.tile_pool(name="ps_mp", bufs=2, space="PSUM") as ps_mp,
        ):
            first_load = [None]

            def _first_load():
                if first_load[0] is not None:
                    first_load[0]()
                    first_load[0] = None

            ident_f = const.tile([128, 128], F32)
            make_identity(nc, ident_f)
            ident_r = const.tile([128, 128], F32R)
            nc.gpsimd.tensor_copy(out=ident_r[:], in_=ident_f[:])
            ones_r = const.tile([1, N], BF16)
            nc.gpsimd.memset(ones_r[:], 1.0)
            at_f = const.tile([D, D], F32)
            atat = const.tile([D, 2, D], F32R)
            hs_f = const.tile([D, D], F32)
            hs_r = const.tile([D, D], F32R)
            bias_t = [const.tile([1, bl * 128], BF16, tag=f"biasR{h}",
                                 name=f"biasR{h}") for h in range(2)]

            def load_consts():
                nc.sync.dma_start(out=at_f[:], in_=at_d[:])
                nc.gpsimd.tensor_copy(out=atat[:, 0, :], in_=at_f[:])
                nc.gpsimd.tensor_copy(out=atat[:, 1, :], in_=at_f[:])
                nc.sync.dma_start(out=hs_f[:], in_=hs_d[:])
                nc.gpsimd.tensor_copy(out=hs_r[:], in_=hs_f[:])
                for h in range(2):
                    nc.sync.dma_start(out=bias_t[h][:], in_=bias_d[h:h + 1, :])

            chunks = [None] * nchunk
            outs = [None] * nchunk
            st1 = [None] * npair   # (diff,)
            st2 = [None] * npair   # (wTs, als)
            st3 = [None] * npair   # (satTs,)
            st4 = [None] * npair   # (pt,)

            def load(it, split=False):
                m_p = mst.tile([128, NBLK, D + 1], BF16, tag="m_p", name="m_p")
                mt_p = mtst.tile([128, CH, N], F32R, tag="mt_p", name="mt_p")
                s_p = sst.tile([128, NBLK, D], F32, tag="s_p", name="s_p")
                c0 = it * NBLK * (D + 1)
                t0 = it * CH * N
                r0 = it * NBLK * 128
                # chunk 0 loads in half-chunk slices, gate/sat inputs first,
                # so the pipeline fills while the rest streams in
                parts = 4 if split else 1
                hb = CH // parts
                for g in range(parts):
                    nc.sync.dma_start(
                        out=mt_p[:, g * hb:(g + 1) * hb, :],
                        in_=mt_d[:, t0 + g * hb * N:t0 + (g + 1) * hb * N].rearrange(
                            "p (b i) -> p b i", b=hb))
                    nc.sync.dma_start(
                        out=s_p[:, g * 2 * hb:(g + 1) * 2 * hb, :],
                        in_=s_d[r0 + g * 2 * hb * 128:r0 + (g + 1) * 2 * hb * 128,
                                :].rearrange("(blk p) d -> p blk d", p=128))
                    nc.sync.dma_start(
                        out=m_p[:, g * 2 * hb:(g + 1) * 2 * hb, :],
                        in_=m_d[:, c0 + g * 2 * hb * (D + 1):
                                c0 + (g + 1) * 2 * hb * (D + 1)].rearrange(
                            "p (blk d) -> p blk d", blk=2 * hb))
                chunks[it] = (m_p, mt_p, s_p)

            def tiles(p):
                c, r = divmod(p, PPC)
                m_p, mt_p, s_p = chunks[c]
                return m_p, mt_p, s_p, outs[c], r * 4, r * 2

            def s1(p):
                m_p, _, s_p, _, blk0, _ = tiles(p)
                diff = wk.tile([128, 4, D], F32, tag="diff", name="diff")
                nc.gpsimd.tensor_tensor(
                    out=diff[:], in0=m_p[:, blk0:blk0 + 4, 0:D],
                    in1=s_p[:, blk0:blk0 + 4, :], op=sub)
                st1[p] = (diff,)

            def s2(p):
                _, mt_p, s_p, _, blk0, bq0 = tiles(p)
                wp = ps_wp.tile([128, 2 * N], F32, tag="wp", name="wp")
                nc.tensor.matmul(
                    wp[:], hs_r[:],
                    mt_p[:, bq0:bq0 + 2, :].rearrange("p a b -> p (a b)"),
                    start=True, stop=True)
                wTs = wk.tile([128, 2 * N], F32R, tag="wTs", name="wTs")
                nc.scalar.copy(out=wTs[:], in_=wp[:])
                vps = []
                for q in range(2):
                    vp = ps_vp.tile([128, 2, 2 * D], F32, tag="vp", name="vp")
                    for h in range(2):
                        nc.tensor.matmul(
                            vp[:, h, :],
                            mt_p[:, bq0 + q, h * 128:(h + 1) * 128],
                            atat[:].rearrange("p a b -> p (a b)"),
                            start=True, stop=True)
                    vps.append(vp)
                dump = wk.tile([128, 4, D], BF16, tag="dump", name="dump")
                als = []
                for k in range(4):
                    q, h = divmod(k, 2)
                    a_t = wk.tile([128, 1], F32, tag=f"al{k}", name=f"al{k}")
                    nc.vector.scalar_tensor_tensor(
                        out=dump[:, k, :], in0=vps[q][:, h, 0:D],
                        scalar=1.0, in1=s_p[:, blk0 + k, :],
                        op0=mult, op1=mult, accum_out=a_t[:])
                    als.append(a_t)
                st2[p] = (wTs, als)

            def s2b(p):
                _, _, s_p, _, blk0, _ = tiles(p)
                (diff,) = st1[p]
                als = st2[p][1]
                c = p // PPC
                sat_p = outs[c][0]
                for k in range(4):
                    nc.vector.scalar_tensor_tensor(
                        out=sat_p[:, blk0 + k, :], in0=diff[:, k, :],
                        scalar=als[k][:], in1=s_p[:, blk0 + k, :],
                        op0=mult, op1=add)

            def s3(p):
                _, _, _, (sat_p, _), blk0, _ = tiles(p)
                tps = ps_tp.tile([128, 2, N], F32R, tag="tp", name="tps")
                for q in range(2):
                    for h in range(2):
                        nc.tensor.transpose(
                            tps[:, q, h * 128:(h + 1) * 128],
                            sat_p[:, blk0 + 2 * q + h, :], ident_r[:])
                satTs = wk.tile([128, 2, N], F32R, tag="satTs", name="satTs")
                nc.scalar.copy(out=satTs[:], in_=tps[:])
                st3[p] = (satTs,)

            def s4(p):
                wTs = st2[p][0]
                (satTs,) = st3[p]
                pt = wk.tile([128, 4, N], BF16, tag="pt", name="pt")
                for q in range(2):
                    b = 2 * p + q
                    bp = ps_bp.tile([128, 2, N], F32, tag="bp", name="bp")
                    for h in range(2):
                        nc.tensor.matmul(
                            bp[:, h, :], satTs[:, q, h * 128:(h + 1) * 128],
                            wTs[:, q * N:(q + 1) * N], start=True, stop=False)
                        nc.tensor.matmul(
                            bp[:, h, :],
                            bias_t[h][:, b * 128:(b + 1) * 128],
                            ones_r[:], start=False, stop=True)
                    nc.scalar.activation(out=pt[:, 2 * q:2 * q + 2, :], in_=bp[:],
                                         func=Exp, scale=1.0)
                st4[p] = (pt,)

            def s5(p):
                m_p, _, _, (_, mir_p), blk0, _ = tiles(p)
                (pt,) = st4[p]
                for q in range(2):
                    b = 2 * p + q
                    mp = ps_mp.tile([128, 2, D + 1], F32, tag="mp", name="mp")
                    for hi in range(2):
                        nc.tensor.matmul(
                            mp[:, hi, :], pt[:, 2 * q, hi * 128:(hi + 1) * 128],
                            m_p[:, blk0 + 2 * q, :], start=True, stop=False)
                        nc.tensor.matmul(
                            mp[:, hi, :], pt[:, 2 * q + 1, hi * 128:(hi + 1) * 128],
                            m_p[:, blk0 + 2 * q + 1, :], start=False, stop=True)
                    if q == 1 and p % 6 == 5:
                        nc.scalar.copy(
                            out=mir_p[:, blk0 + 2 * q:blk0 + 2 * q + 2, :],
                            in_=mp[:])
                    else:
                        nc.vector.tensor_copy(
                            out=mir_p[:, blk0 + 2 * q:blk0 + 2 * q + 2, :],
                            in_=mp[:])

            def store_sat(c, r=None):
                sat_p = outs[c][0]
                if r is None:
                    c0 = c * NBLK * D
                    nc.gpsimd.dma_start(
                        out=sat_d[:, c0:c0 + NBLK * D].rearrange(
                            "p (blk d) -> p blk d", blk=NBLK),
                        in_=sat_p[:])
                else:
                    c0 = (c * NBLK + r * 4) * D
                    nc.gpsimd.dma_start(
                        out=sat_d[:, c0:c0 + 4 * D].rearrange(
                            "p (blk d) -> p blk d", blk=4),
                        in_=sat_p[:, r * 4:(r + 1) * 4, :])

            def store_mir(c, r=None):
                mir_p = outs[c][1]
                if r is None:
                    c0 = c * NBLK * (D + 1)
                    nc.sync.dma_start(
                        out=mir_d[:, c0:c0 + NBLK * (D + 1)].rearrange(
                            "p (blk d) -> p blk d", blk=NBLK),
                        in_=mir_p[:])
                else:
                    c0 = (c * NBLK + r * 4) * (D + 1)
                    nc.sync.dma_start(
                        out=mir_d[:, c0:c0 + 4 * (D + 1)].rearrange(
                            "p (blk d) -> p blk d", blk=4),
                        in_=mir_p[:, r * 4:(r + 1) * 4, :])

            load_consts()
            load(0, split=True)
            if nchunk > 1:
                load(1)
            for i in range(npair + 5):
                if i < npair:
                    c, r = divmod(i, PPC)
                    if r == 0:
                        sat_p = ost.tile([128, NBLK, D], F32R, tag="sat_p",
                                         name="sat_p")
                        mir_p = ost.tile([128, NBLK, D + 1], BF16, tag="mir_p",
                                         name="mir_p")
                        outs[c] = (sat_p, mir_p)
                    if r == 1 and c + 2 < nchunk:
                        load(c + 2)
                    s1(i)
                if 0 <= i - 1 < npair:
                    s2(i - 1)
                if 0 <= i - 2 < npair:
                    s2b(i - 2)
                    c2, r2 = divmod(i - 2, PPC)
                    if c2 == nchunk - 1:
                        store_sat(c2, r2)
                    elif r2 == PPC - 1:
                        store_sat(c2)
                if 0 <= i - 3 < npair:
                    s3(i - 3)
                if 0 <= i - 4 < npair:
                    s4(i - 4)
                if 0 <= i - 5 < npair:
                    s5(i - 5)
                    c5, r5 = divmod(i - 5, PPC)
                    if c5 == nchunk - 1:
                        store_mir(c5, r5)
                    elif r5 == PPC - 1:
                        store_mir(c5)
    nc.finalize()
    return nc


def _get_nc():
    if "nc" not in _CACHE:
        _CACHE["nc"] = _build()
    return _CACHE["nc"]


def _pack_img(a, cols):
    """[ROWS, cols] -> SBUF image [128, NCHUNK*NBLK*cols]: row
    r=(c*NBLK+blk)*128+p lands at partition p, columns ((c*NBLK)+blk)*cols."""
    return np.ascontiguousarray(
        a.reshape(NCHUNK, NBLK, 128, cols).transpose(2, 0, 1, 3).reshape(
            128, NCHUNK * NBLK * cols))


def _unpack_img(img, cols):
    """Inverse of _pack_img."""
    return np.ascontiguousarray(
        img.reshape(128, NCHUNK, NBLK, cols).transpose(1, 2, 0, 3).reshape(
            ROWS, cols))


def run(inputs, trace=False, **kw):
    mirror = np.ascontiguousarray(np.asarray(inputs["mirror_nodes"], dtype=np.float32))
    sat = np.ascontiguousarray(np.asarray(inputs["satellite_nodes"], dtype=np.float32))
    mask = np.asarray(inputs["satellite_node_mask"])
    Wq1 = np.asarray(inputs["Wq1"], dtype=np.float64)
    Wk1 = np.asarray(inputs["Wk1"], dtype=np.float64)
    Wq2 = np.asarray(inputs["Wq2"], dtype=np.float64)
    Wk2 = np.asarray(inputs["Wk2"], dtype=np.float64)

    scale = 1.0 / math.sqrt(D)
    At = np.ascontiguousarray((scale * (Wk1.T @ Wq1)).astype(np.float32))
    Hs = np.ascontiguousarray((scale * (Wq2.T @ Wk2)).astype(np.float32))

    m_bf = mirror.reshape(B * N, D).astype(BF)
    m1 = np.empty((B * N, D + 1), dtype=BF)
    m1[:, 0:D] = m_bf
    m1[:, D] = np.ones((), dtype=BF)

    nc = _get_nc()
    in_maps = []
    for c in range(NCORES):
        lo, hi = c * BL, (c + 1) * BL
        # biasR[h, b*128 + j] = 0 if mask[b, 128h + j] else -1e30
        bias = np.where(mask[lo:hi], 0.0, NEG).astype(BF)          # [BL, N]
        biasR = np.ascontiguousarray(
            bias.reshape(BL, 2, 128).transpose(1, 0, 2).reshape(2, BL * 128))
        # mt_img[d, b*N + i] = m[b, i, d]
        mt = np.ascontiguousarray(
            mirror[lo:hi].transpose(2, 0, 1).reshape(128, BL * N))
        in_maps.append({
            "m_img": _pack_img(m1[lo * N:hi * N], D + 1),
            "mt_img": mt,
            "s": sat[lo:hi].reshape(ROWS, D),
            "biasR": biasR,
            "At": At,
            "Hs": Hs,
        })
    res = run_bass_kernel_spmd(nc, in_maps, list(range(NCORES)), trace=trace, **kw)
    sat_out = np.concatenate(
        [_unpack_img(np.asarray(r["sat_img"]), D).astype(np.float32).reshape(BL, N, D)
         for r in res.results], axis=0)
    mirs = []
    for r in res.results:
        nd = _unpack_img(np.asarray(r["mir_img"]), D + 1).astype(
            np.float32).reshape(BL, N, D + 1)
        mirs.append(nd[:, :, 0:D] / nd[:, :, D:D + 1])
    mir_out = np.concatenate(mirs, axis=0)
    return (sat_out, mir_out), res


def kernel(**inputs):
    out, _ = run(inputs)
    return out
